# revision 14
# baseline (speedup 1.0000x reference)
"""Trainium2 Bass kernel for nn_CrossLayerLight (cross-cloud KNN message passing).

Sharding: 8 cores = 2 directions x 2 batches x 2 query-halves.
Each core: 4096 queries vs 8192 candidates.

Round-3 design: Morton-sorted queries and candidates; candidates in 64
Morton-contiguous groups of 128.  Per 128-query tile the device selects the
top-16 groups by a coarse bound (-(d(q,ctr_g)-r_g)^2 maxed over the tile's
queries, computed once for all tiles in phase C0), gathers those groups'
score-table rows (2048 candidates), and runs exact top-16 on the 2048-wide
window only.  On this dataset each tile's true top-16 neighbors span <= 8
groups, so 16 give 2x margin (validated offline).

Per-core device pipeline:
  A) v-table build: v[j] = feat2[j] + xyz2[j] @ pos_w^T -> bf16 hi/lo pair
     rows [8192, 128] in DRAM for gathering
  B) u-table build: u[q] = feat1[q] - xyz1[q] @ pos_w^T + pos_b
  C0) coarse group stage: d2(q, ctr_g) via 32-row bf16 matmul, sqrt (ACT),
      lb = -relu(d-r)^2, priority = partition-all-reduce max (GPSIMD),
      per-tile top-16 groups + ids (DVE max8/max_index)
  C) per 128-query tile:
     - gather the 16 selected groups' score-table rows (256B each) ->
       [128p, 2048] bf16 rhs; scores via 30-row bf16 matmul (PE, ~1e-6)
     - exact top-16 of the 2048 window: ACT copies PSUM->SBUF, then DVE
       max8/max_index/match_replace/max8/max_index (window positions)
     - window pos -> global candidate id: id = g16[w>>7]*128 + (w&127) with
       the 16-entry group lookup done by a one-hot multiply-reduce
     - idx transpose+replicate via PE transpose -> dma_gather of v-pairs
     - z0/L0 relu pairs fused via output-stacked weights; L1 -> max-pool over
       k from PSUM (DVE); final 64->128 linear with fused bias (PE), DMA out.
"""

import sys
import numpy as np
import ml_dtypes

sys.path.insert(0, "/opt/trn_rl_repo")

import concourse.bacc as bacc  # noqa: E402
import concourse.mybir as mybir  # noqa: E402
import concourse.bass_isa as bass_isa  # noqa: E402
from concourse.bass_utils import run_bass_kernel_spmd  # noqa: E402
from concourse.tile import TileContext  # noqa: E402

BF16 = ml_dtypes.bfloat16
F32 = mybir.dt.float32
F32R = mybir.dt.float32r
BF = mybir.dt.bfloat16
U32 = mybir.dt.uint32
I16 = mybir.dt.int16

NQ_TOT = 4096   # queries per core
NCAND = 8192    # candidates per core
D = 64          # feature dim
KNN = 16
NROW = 30       # score matmul contraction rows
NGRP = 64       # candidate groups (128 each)
GSEL = 16       # groups selected per query tile -> window of 2048
WSEL = GSEL * 128
LEAKY = 0.1
MBITS = 10

_CACHE = {}


def build_nc(nq_tot=NQ_TOT, ncand=NCAND, body_repeats=1):
    nc = bacc.Bacc()
    ntiles = nq_tot // 128

    # ---- external inputs ----
    sc_lhsT = nc.dram_tensor("sc_lhsT", [NROW, nq_tot], BF, kind="ExternalInput")
    scotab = nc.dram_tensor("scotab", [ncand, 128], BF, kind="ExternalInput")
    cg_lhsT = nc.dram_tensor("cg_lhsT", [34, nq_tot], BF, kind="ExternalInput")
    cg_rhs = nc.dram_tensor("cg_rhs", [34, NGRP], BF, kind="ExternalInput")
    rg_in = nc.dram_tensor("rg_in", [128, NGRP], F32, kind="ExternalInput")
    vb_lhsT = nc.dram_tensor("vb_lhsT", [67, ncand], F32, kind="ExternalInput")
    vb_rhs = nc.dram_tensor("vb_rhs", [67, D], F32, kind="ExternalInput")
    ub_lhsT = nc.dram_tensor("ub_lhsT", [68, D], F32, kind="ExternalInput")
    ub_rhs = nc.dram_tensor("ub_rhs", [68, nq_tot], F32, kind="ExternalInput")
    w0T = nc.dram_tensor("w0T", [2 * D, 2 * D], F32R, kind="ExternalInput")
    w1T = nc.dram_tensor("w1T", [2 * D, D], F32R, kind="ExternalInput")
    bb0 = nc.dram_tensor("bb0", [2 * D, 1], F32, kind="ExternalInput")
    b1c = nc.dram_tensor("b1c", [D, 1], F32, kind="ExternalInput")
    b1n = nc.dram_tensor("b1n", [D, 1], F32, kind="ExternalInput")
    t_rhs = nc.dram_tensor("t_rhs", [2 * D, 128], F32R, kind="ExternalInput")
    tb_row = nc.dram_tensor("tb_row", [1, 128], F32, kind="ExternalInput")
    ii128 = nc.dram_tensor("ii128", [2 * D, 2 * D], BF, kind="ExternalInput")
    id128u = nc.dram_tensor("id128u", [128, 128], F32, kind="ExternalInput")
    gpat_in = nc.dram_tensor("gpat_in", [128, 128], F32, kind="ExternalInput")
    iota_in = nc.dram_tensor("iota_in", [128, GSEL], F32, kind="ExternalInput")
    shc_in = nc.dram_tensor("shc_in", [128, 4], U32, kind="ExternalInput")

    out = nc.dram_tensor("out", [nq_tot, 128], F32, kind="ExternalOutput")

    with TileContext(nc) as tc:
        with (
            tc.tile_pool(name="const", bufs=1) as cst,
            tc.tile_pool(name="dram", bufs=1, space="DRAM") as dram,
        ):
            vpair = dram.tile([ncand, 128], BF)

            # persistent SBUF tiles
            sc_l = cst.tile([NROW, nq_tot], BF)
            cgl = cst.tile([34, nq_tot], BF)
            cgr = cst.tile([34, NGRP], BF)
            rgs = cst.tile([128, NGRP], F32)
            uhl = cst.tile([128, nq_tot], BF)      # rows 0:64 u_hi, 64:128 u_lo
            w0s = cst.tile([2 * D, 2 * D], F32R)
            w1s = cst.tile([2 * D, D], F32R)
            bb0s = cst.tile([2 * D, 1], F32)
            b1s = cst.tile([D, 1], F32)
            b1ns = cst.tile([D, 1], F32)
            trs = cst.tile([2 * D, 128], F32R)
            tbs = cst.tile([1, 128], F32)
            ones1 = cst.tile([1, 128], F32)
            iis = cst.tile([2 * D, 2 * D], BF)
            idu = cst.tile([128, 128], F32)
            gpatc = cst.tile([128, 128], F32)
            iotac = cst.tile([128, GSEL], F32)
            shc = cst.tile([128, 4], U32)          # cols: [7, 127, 0, 0]
            g16all = cst.tile([128, ntiles * GSEL], F32)
            for dst, src in [(sc_l, sc_lhsT), (cgl, cg_lhsT), (cgr, cg_rhs),
                             (rgs, rg_in), (w0s, w0T), (w1s, w1T), (bb0s, bb0),
                             (b1s, b1c), (b1ns, b1n), (trs, t_rhs), (tbs, tb_row),
                             (iis, ii128), (idu, id128u), (gpatc, gpat_in),
                             (iotac, iota_in), (shc, shc_in)]:
                nc.sync.dma_start(out=dst[:], in_=src[:])
            nc.vector.memset(ones1[:], 1.0)

            for _body_rep in range(body_repeats):
                # ---- phase A: v table ----
                with (
                    tc.tile_pool(name="phA", bufs=2) as pha,
                    tc.tile_pool(name="phA_ps", bufs=2, space="PSUM") as phaps,
                ):
                    vbw = pha.tile([67, D], F32, tag="vbw")
                    nc.sync.dma_start(out=vbw[:], in_=vb_rhs[:])
                    njt = ncand // 128
                    grp = 8  # j-tiles per psum fill
                    for g in range(njt // grp):
                        pv = phaps.tile([128, grp * D], F32, tag="pv")
                        for s in range(grp):
                            jt = g * grp + s
                            vbl = pha.tile([67, 128], F32, tag="vbl")
                            nc.sync.dma_start(out=vbl[:], in_=vb_lhsT[:, jt * 128:(jt + 1) * 128])
                            nc.tensor.matmul(pv[:, s * D:(s + 1) * D], vbl[:], vbw[:],
                                             start=True, stop=True)
                        vhi = pha.tile([128, grp * D], BF, tag="vhi")
                        vlo = pha.tile([128, grp * D], BF, tag="vlo")
                        nc.scalar.activation(vhi[:], pv[:], mybir.ActivationFunctionType.Copy)
                        nc.vector.tensor_sub(vlo[:], pv[:], vhi[:])
                        dst = vpair[g * grp * 128:(g + 1) * grp * 128, :]
                        dst_hi = dst[:, 0:D].rearrange("(s p) f -> p s f", p=128)
                        dst_lo = dst[:, D:128].rearrange("(s p) f -> p s f", p=128)
                        nc.sync.dma_start(out=dst_hi, in_=vhi[:].rearrange("p (s f) -> p s f", f=D))
                        nc.sync.dma_start(out=dst_lo, in_=vlo[:].rearrange("p (s f) -> p s f", f=D))

                # ---- phase B: u table ----
                with (
                    tc.tile_pool(name="phB", bufs=2) as phb,
                    tc.tile_pool(name="phB_ps", bufs=2, space="PSUM") as phbps,
                ):
                    ubw = phb.tile([68, D], F32, tag="ubw")
                    nc.sync.dma_start(out=ubw[:], in_=ub_lhsT[:])
                    uchunk = min(2048, nq_tot)
                    for h in range(nq_tot // uchunk):
                        ur = phb.tile([68, uchunk], F32, tag="ur")
                        nc.sync.dma_start(out=ur[:], in_=ub_rhs[:, h * uchunk:(h + 1) * uchunk])
                        pu = phbps.tile([D, uchunk], F32, tag="pu")
                        for j in range(uchunk // 512 or 1):
                            w = min(512, uchunk)
                            nc.tensor.matmul(pu[:, j * w:(j + 1) * w], ubw[:],
                                             ur[:, j * w:(j + 1) * w], start=True, stop=True)
                        nc.scalar.activation(uhl[0:D, h * uchunk:(h + 1) * uchunk], pu[:],
                                             mybir.ActivationFunctionType.Copy)
                        nc.vector.tensor_sub(uhl[D:128, h * uchunk:(h + 1) * uchunk], pu[:],
                                             uhl[0:D, h * uchunk:(h + 1) * uchunk])

                # ---- phase C ----
                with (
                    tc.tile_pool(name="wk", bufs=2) as wk,
                    tc.tile_pool(name="cw", bufs=1) as cw,
                    tc.tile_pool(name="ps_sc", bufs=2, space="PSUM") as pssc,
                    tc.tile_pool(name="ps_z", bufs=2, space="PSUM") as psz,
                    tc.tile_pool(name="ps_tr", bufs=1, space="PSUM") as pstr,
                    tc.tile_pool(name="ps_t1", bufs=1, space="PSUM") as pst1,
                ):
                    # C0: coarse group stage for all tiles
                    dall = cw.tile([128, ntiles * NGRP], F32)   # d(q, ctr_g)
                    nlb = cw.tile([128, ntiles * NGRP], F32)
                    for half in range(2):
                        pst = pssc.tile([128, 1024], F32, tag="psc")
                        for tt in range(16):
                            t = half * 16 + tt
                            nc.tensor.matmul(pst[:, tt * NGRP:(tt + 1) * NGRP],
                                             cgl[:, t * 128:(t + 1) * 128], cgr[:],
                                             start=True, stop=True)
                        nc.scalar.activation(dall[:, half * 1024:(half + 1) * 1024],
                                             pst[:], mybir.ActivationFunctionType.Sqrt)
                    rgb = rgs[:].unsqueeze(1).to_broadcast([128, ntiles, NGRP])
                    d3 = dall[:].rearrange("p (t g) -> p t g", g=NGRP)
                    nlb3 = nlb[:].rearrange("p (t g) -> p t g", g=NGRP)
                    nc.vector.tensor_tensor(out=nlb3, in0=d3, in1=rgb,
                                            op=mybir.AluOpType.subtract)
                    nc.vector.tensor_scalar(nlb[:], nlb[:], 0.0, scalar2=None,
                                            op0=mybir.AluOpType.max)
                    nc.vector.scalar_tensor_tensor(
                        out=nlb[:], in0=nlb[:], scalar=-1.0, in1=nlb[:],
                        op0=mybir.AluOpType.mult, op1=mybir.AluOpType.mult)
                    nc.gpsimd.partition_all_reduce(out_ap=nlb[:], in_ap=nlb[:],
                                                   channels=128,
                                                   reduce_op=bass_isa.ReduceOp.max)
                    g8t = cw.tile([128, 16], F32)
                    gpos = cw.tile([128, GSEL], U32)
                    for t in range(ntiles):
                        sl_n = nlb[:, t * NGRP:(t + 1) * NGRP]
                        nc.vector.max(out=g8t[:, 0:8], in_=sl_n)
                        nc.vector.max_index(out=gpos[:, 0:8], in_max=g8t[:, 0:8],
                                            in_values=sl_n)
                        nc.vector.match_replace(out=sl_n, in_to_replace=g8t[:, 0:8],
                                                in_values=sl_n, imm_value=-3.0e38)
                        nc.vector.max(out=g8t[:, 8:16], in_=sl_n)
                        nc.vector.max_index(out=gpos[:, 8:16], in_max=g8t[:, 8:16],
                                            in_values=sl_n)
                        nc.vector.tensor_copy(g16all[:, t * GSEL:(t + 1) * GSEL],
                                              gpos[:])

                    # C per tile
                    for qt in range(ntiles):
                        q0 = qt * 128
                        g16f = g16all[:, qt * GSEL:(qt + 1) * GSEL]
                        # window gather: idx[p,c] = g16[c>>3]*128 + 16*(c&7) + p%16
                        gA = wk.tile([128, 128], F32, tag="gA")
                        nc.vector.tensor_copy(
                            gA[:].rearrange("p (a b) -> p a b", b=8),
                            g16f.unsqueeze(2).to_broadcast([128, GSEL, 8]))
                        idst = wk.tile([128, 128], F32, tag="idst")
                        nc.vector.scalar_tensor_tensor(
                            out=idst[:], in0=gA[:], scalar=128.0, in1=gpatc[:],
                            op0=mybir.AluOpType.mult, op1=mybir.AluOpType.add)
                        idxs2 = wk.tile([128, 128], I16, tag="idxs2")
                        nc.vector.tensor_copy(idxs2[:], idst[:])
                        srhs = wk.tile([128, 1, WSEL], BF, tag="srhs")
                        nc.gpsimd.dma_gather(out_ap=srhs[:], in_ap=scotab[:],
                                             idxs_ap=idxs2[:], num_idxs=WSEL,
                                             num_idxs_reg=WSEL, elem_size=128,
                                             transpose=True, single_packet=False)
                        # scores on the window
                        wsb = wk.tile([128, WSEL], F32, tag="wsb")
                        for b2 in range(WSEL // 1024):
                            pst = pssc.tile([128, 1024], F32, tag="psc")
                            for j in range(2):
                                c0 = b2 * 1024 + j * 512
                                nc.tensor.matmul(pst[:, j * 512:(j + 1) * 512],
                                                 sc_l[:, q0:q0 + 128],
                                                 srhs[0:NROW, 0, c0:c0 + 512],
                                                 start=True, stop=True)
                            nc.scalar.activation(wsb[:, b2 * 1024:(b2 + 1) * 1024],
                                                 pst[:], mybir.ActivationFunctionType.Copy)
                        # exact top-16 of the window (positions = window idx)
                        r16 = wk.tile([128, 16], F32, tag="r16")
                        pos = wk.tile([128, 16], U32, tag="pos")
                        nc.vector.max(out=r16[:, 0:8], in_=wsb[:])
                        nc.vector.max_index(out=pos[:, 0:8], in_max=r16[:, 0:8],
                                            in_values=wsb[:])
                        nc.vector.match_replace(out=wsb[:], in_to_replace=r16[:, 0:8],
                                                in_values=wsb[:], imm_value=-3.0e38)
                        nc.vector.max(out=r16[:, 8:16], in_=wsb[:])
                        nc.vector.max_index(out=pos[:, 8:16], in_max=r16[:, 8:16],
                                            in_values=wsb[:])
                        # window pos -> global candidate id
                        gidx = wk.tile([128, 16], U32, tag="gidx")
                        rem = wk.tile([128, 16], U32, tag="rem")
                        nc.vector.tensor_scalar(gidx[:], pos[:], shc[:, 0:1],
                                                scalar2=None,
                                                op0=mybir.AluOpType.logical_shift_right)
                        nc.vector.tensor_scalar(rem[:], pos[:], shc[:, 1:2],
                                                scalar2=None,
                                                op0=mybir.AluOpType.bitwise_and)
                        gf = wk.tile([128, 16], F32, tag="gf")
                        remf = wk.tile([128, 16], F32, tag="remf")
                        nc.vector.tensor_copy(gf[:], gidx[:])
                        nc.vector.tensor_copy(remf[:], rem[:])
                        eqm = wk.tile([128, 16, GSEL], F32, tag="eqm")
                        nc.vector.tensor_tensor(
                            out=eqm[:],
                            in0=gf[:].unsqueeze(2).to_broadcast([128, 16, GSEL]),
                            in1=iotac[:].unsqueeze(1).to_broadcast([128, 16, GSEL]),
                            op=mybir.AluOpType.is_equal)
                        nc.vector.tensor_tensor(
                            out=eqm[:], in0=eqm[:],
                            in1=g16f.unsqueeze(1).to_broadcast([128, 16, GSEL]),
                            op=mybir.AluOpType.mult)
                        gbase = wk.tile([128, 16], F32, tag="gbase")
                        nc.vector.tensor_reduce(out=gbase[:], in_=eqm[:],
                                                axis=mybir.AxisListType.X,
                                                op=mybir.AluOpType.add)
                        idxf = wk.tile([128, 16], F32, tag="idxf")
                        nc.vector.scalar_tensor_tensor(
                            out=idxf[:], in0=gbase[:], scalar=128.0, in1=remf[:],
                            op0=mybir.AluOpType.mult, op1=mybir.AluOpType.add)
                        # C3: idx -> replicate x8 in free dim -> PE transpose
                        i16r = wk.tile([128, 128], F32, tag="i16r")
                        rep = idxf[:].unsqueeze(1).to_broadcast([128, 8, 16])
                        nc.vector.tensor_copy(i16r[:].rearrange("p (r k) -> p r k", k=16), rep)
                        ptr = pstr.tile([128, 128], F32, tag="tr")
                        nc.tensor.transpose(ptr[:], i16r[:], idu[:])
                        idxs = wk.tile([128, 128], I16, tag="idxs")
                        nc.vector.tensor_copy(idxs[:], ptr[:])
                        # C4: gather v pairs -> [128, 2048] bf16 (cols q*16+k)
                        gt = wk.tile([128, 1, 2048], BF, tag="gt")
                        nc.gpsimd.dma_gather(out_ap=gt[:], in_ap=vpair[:], idxs_ap=idxs[:],
                                             num_idxs=2048, num_idxs_reg=2048,
                                             elem_size=128, transpose=True,
                                             single_packet=False)
                        gtf = gt[:].rearrange("p a n -> p (a n)")
                        # C5-C8 per 512-col block (32 queries): z0/L0 emit [y; -y]
                        pooled = wk.tile([D, 128], F32, tag="pooled")
                        for cb in range(4):
                            sl = slice(cb * 512, (cb + 1) * 512)
                            pz0 = psz.tile([128, 512], F32, tag="pz")
                            nc.tensor.matmul(pz0[:], iis[:], gtf[:, sl],
                                             start=True, stop=False)
                            urhs = uhl[:, q0 + cb * 32:q0 + (cb + 1) * 32] \
                                .unsqueeze(2).to_broadcast([128, 32, KNN])
                            nc.tensor.matmul(pz0[:].rearrange("p (q k) -> p q k", k=KNN),
                                             iis[:], urhs, start=False, stop=True)
                            rp0 = wk.tile([2 * D, 512], F32R, tag="rp")
                            nc.scalar.activation(rp0[:], pz0[:],
                                                 mybir.ActivationFunctionType.Relu)
                            pz1 = psz.tile([128, 512], F32, tag="pz")
                            nc.tensor.matmul(pz1[:], w0s[:], rp0[:], start=True, stop=True)
                            rp1 = wk.tile([2 * D, 512], F32R, tag="rp")
                            nc.scalar.activation(rp1[:], pz1[:],
                                                 mybir.ActivationFunctionType.Relu,
                                                 bias=bb0s[:])
                            pz2 = psz.tile([D, 512], F32, tag="pz")
                            nc.tensor.matmul(pz2[:], w1s[:], rp1[:], start=True, stop=True)
                            nc.vector.tensor_reduce(
                                out=pooled[:, cb * 32:(cb + 1) * 32],
                                in_=pz2[:].rearrange("p (q k) -> p q k", k=KNN),
                                axis=mybir.AxisListType.X, op=mybir.AluOpType.max)
                        # t-linear
                        tl = wk.tile([2 * D, 128], F32R, tag="tl")
                        nc.scalar.activation(tl[0:D, :], pooled[:],
                                             mybir.ActivationFunctionType.Relu, bias=b1s[:])
                        nc.scalar.activation(tl[D:2 * D, :], pooled[:],
                                             mybir.ActivationFunctionType.Relu,
                                             bias=b1ns[:], scale=-1.0)
                        pt1 = pst1.tile([128, 128], F32, tag="t1")
                        nc.tensor.matmul(pt1[:], tl[:], trs[:], start=True, stop=False)
                        nc.tensor.matmul(pt1[:], ones1[:], tbs[:], start=False, stop=True)
                        outt = wk.tile([128, 128], F32, tag="outt")
                        nc.scalar.activation(outt[:], pt1[:], mybir.ActivationFunctionType.Copy)
                        nc.sync.dma_start(out=out[q0:q0 + 128, :], in_=outt[:])

    nc.compile()
    return nc


def _split_bf16(x, n):
    parts = []
    rem = np.asarray(x, np.float64)
    for _ in range(n):
        p = rem.astype(BF16)
        parts.append(p)
        rem = rem - p.astype(np.float64)
    return parts


def _morton_key(cand, pts):
    n = len(cand)
    q = np.zeros((len(pts), 3), np.uint64)
    for c in range(3):
        sc = np.sort(cand[:, c])
        q[:, c] = np.clip(np.searchsorted(sc, pts[:, c]) * (1 << MBITS) // (n + 1),
                          0, (1 << MBITS) - 1)
    k = np.zeros(len(pts), np.uint64)
    for b in range(MBITS):
        for c in range(3):
            k |= ((q[:, c] >> np.uint64(b)) & np.uint64(1)) << np.uint64(3 * b + c)
    return k


def _score_rows(qxyz, cxyz, negate_c=False, norm_sign=-1.0, norm_of="c"):
    """30-row bf16 3-term-split factorization of 2 q.p + norm_sign*|p|^2."""
    nq = qxyz.shape[0]
    A = _split_bf16(2.0 * qxyz, 3)
    P = _split_bf16(cxyz, 3)
    nrm = np.sum(np.asarray(cxyz, np.float64) ** 2, -1)
    m = _split_bf16(norm_sign * nrm, 3)
    rows_q, rows_c = [], []
    prods = sorted(((i, j) for i in range(3) for j in range(3)),
                   key=lambda t: -(t[0] + t[1]))
    sgn = -1.0 if negate_c else 1.0
    for (i, j) in prods:
        for c in range(3):
            rows_q.append(A[i][:, c])
            rows_c.append(sgn * P[j][:, c])
    ones = np.ones(nq, BF16)
    for t in (m[2], m[1], m[0]):
        rows_q.append(ones)
        rows_c.append(t)
    return np.stack(rows_q).astype(BF16), np.stack(rows_c).astype(BF16)


def prep_core_inputs(qxyz, qfeat, cxyz, cfeat, pos_w, pos_b, tw, tb):
    """Per-core input map. Host work is O(N log N) sort + O(N*small) layout."""
    # Morton sort both clouds along the candidate-cloud quantile curve
    cperm = np.argsort(_morton_key(cxyz, cxyz), kind="stable")
    qperm = np.argsort(_morton_key(cxyz, qxyz), kind="stable")
    qxyz, qfeat = qxyz[qperm], qfeat[qperm]
    cxyz, cfeat = cxyz[cperm], cfeat[cperm]
    nq = qxyz.shape[0]
    ncand = cxyz.shape[0]

    sc_lhsT, sc_rhs = _score_rows(qxyz, cxyz)    # [30, nq], [30, ncand]
    scotab = np.zeros((ncand, 128), BF16)
    scotab[:, 0:NROW] = sc_rhs.T

    # groups: bbox centers + radii
    cg = cxyz.reshape(NGRP, 128, 3).astype(np.float64)
    ctr = (cg.min(1) + cg.max(1)) / 2                       # [G,3]
    rad = np.sqrt(((cg - ctr[:, None, :]) ** 2).sum(-1)).max(1)  # [G]

    # coarse d2 matmul: d2 = |q|^2 - 2 q.c + |c|^2
    cq_rows, cc_rows = _score_rows(qxyz, ctr, negate_c=True, norm_sign=1.0)
    q2 = _split_bf16(np.sum(qxyz.astype(np.float64) ** 2, -1), 3)
    onesq = np.ones(nq, BF16)
    onesg = np.ones(NGRP, BF16)
    eps_row = np.full(NGRP, 1e-5, BF16)
    cg_lhsT = np.concatenate([cq_rows, np.stack([q2[0], q2[1], q2[2], onesq])]).astype(BF16)
    cg_rhs = np.concatenate([cc_rows, np.stack([onesg, onesg, onesg, eps_row])]).astype(BF16)
    rg_in = np.broadcast_to(rad.astype(np.float32), (128, NGRP)).copy()

    vb_lhsT = np.concatenate([cxyz.T, cfeat.T]).astype(np.float32)       # [67, ncand]
    vb_rhs = np.concatenate([pos_w.T, np.eye(D)]).astype(np.float32)     # [67, 64]
    ub_lhsT = np.concatenate([-pos_w.T, np.eye(D), pos_b[None, :]]).astype(np.float32)
    ub_rhs = np.concatenate([qxyz.T, qfeat.T, np.ones((1, nq))]).astype(np.float32)

    t_rhs = np.concatenate([tw.T, -LEAKY * tw.T]).astype(np.float32)     # [128, 128]
    tb_row = tb[None, :].astype(np.float32)
    ii1 = np.concatenate([np.eye(D), np.eye(D)])                         # [128, 64]
    ii = np.concatenate([ii1, -ii1], axis=1).astype(BF16)                # [128, 128]
    idu = np.eye(128).astype(np.float32)

    # gather idx pattern: gpat[p, c] = 16*(c&7) + p%16
    pp = (np.arange(128) % 16)[:, None]
    cc = (np.arange(128) & 7)[None, :] * 16
    gpat = (pp + cc).astype(np.float32)
    iota = np.broadcast_to(np.arange(GSEL, dtype=np.float32), (128, GSEL)).copy()
    shc = np.broadcast_to(np.array([7, 127, 0, 0], np.uint32), (128, 4)).copy()

    return {
        "sc_lhsT": sc_lhsT, "scotab": scotab,
        "cg_lhsT": cg_lhsT, "cg_rhs": cg_rhs, "rg_in": rg_in,
        "vb_lhsT": vb_lhsT, "vb_rhs": vb_rhs,
        "ub_lhsT": ub_lhsT, "ub_rhs": ub_rhs,
        "w0T": None, "w1T": None, "bb0": None,  # filled by caller (shared)
        "b1c": None, "b1n": None,
        "t_rhs": t_rhs, "tb_row": tb_row, "ii128": ii, "id128u": idu,
        "gpat_in": gpat, "iota_in": iota, "shc_in": shc,
        "_qperm": qperm,
    }


def build_in_maps(inputs):
    pc1 = np.asarray(inputs["pc1"]); pc2 = np.asarray(inputs["pc2"])
    feat1 = np.asarray(inputs["feat1"]); feat2 = np.asarray(inputs["feat2"])
    pos_w = np.asarray(inputs["pos_w"]); pos_b = np.asarray(inputs["pos_b"])
    w0 = np.asarray(inputs["mlp_w0"]); b0 = np.asarray(inputs["mlp_b0"])
    w1 = np.asarray(inputs["mlp_w1"]); b1 = np.asarray(inputs["mlp_b1"])
    t1w = np.asarray(inputs["t1_w"]); t1b = np.asarray(inputs["t1_b"])
    t2w = np.asarray(inputs["t2_w"]); t2b = np.asarray(inputs["t2_b"])

    w0a = np.concatenate([w0.T, -LEAKY * w0.T])                  # [128, 64]
    w0T = np.concatenate([w0a, -w0a], axis=1).astype(np.float32)  # [128, 128]
    w1T = np.concatenate([w1.T, -LEAKY * w1.T]).astype(np.float32)
    bb0 = np.concatenate([b0, -b0]).astype(np.float32)[:, None].copy()
    b1c = b1.astype(np.float32)[:, None].copy()

    half = NQ_TOT
    in_maps = []
    core_meta = []
    for d in range(2):
        for b in range(2):
            for h in range(2):
                if d == 0:
                    q, p, fq, fp, tw, tb = pc1[b], pc2[b], feat1[b], feat2[b], t1w, t1b
                else:
                    q, p, fq, fp, tw, tb = pc2[b], pc1[b], feat2[b], feat1[b], t2w, t2b
                sl = slice(h * half, (h + 1) * half)
                m = prep_core_inputs(q[sl], fq[sl], p, fp, pos_w, pos_b, tw, tb)
                m["w0T"] = w0T; m["w1T"] = w1T; m["bb0"] = bb0
                m["b1c"] = b1c; m["b1n"] = -b1c
                qperm = m.pop("_qperm")
                in_maps.append(m)
                core_meta.append((d, b, h, qperm))
    return in_maps, core_meta


def kernel(pc1, pc2, feat1, feat2, pos_w, pos_b, mlp_w0, mlp_b0,
           mlp_w1, mlp_b1, t1_w, t1_b, t2_w, t2_b, _trace=False):
    pc1 = np.asarray(pc1)

    if "nc" not in _CACHE:
        _CACHE["nc"] = build_nc()
    nc = _CACHE["nc"]

    inputs = dict(pc1=pc1, pc2=pc2, feat1=feat1, feat2=feat2, pos_w=pos_w,
                  pos_b=pos_b, mlp_w0=mlp_w0, mlp_b0=mlp_b0, mlp_w1=mlp_w1,
                  mlp_b1=mlp_b1, t1_w=t1_w, t1_b=t1_b, t2_w=t2_w, t2_b=t2_b)
    in_maps, core_meta = build_in_maps(inputs)
    _CACHE["last_in_maps"] = in_maps

    res = run_bass_kernel_spmd(nc, in_maps, core_ids=list(range(8)), trace=_trace)
    _CACHE["last_res"] = res
    half = NQ_TOT

    B, N = pc1.shape[0], pc1.shape[1]
    f1 = np.zeros((B, N, 128), np.float32)
    f2 = np.zeros((B, N, 128), np.float32)
    for (dd, b, h, qperm), r in zip(core_meta, res.results):
        o = r["out"]
        tgt = f1 if dd == 0 else f2
        tgt[b, h * half + qperm, :] = o
    return f1, f2


if __name__ == "__main__":
    rng = np.random.default_rng(0)
    B, N = 2, 8192
    ins = {
        "pc1": rng.standard_normal((B, N, 3), np.float32),
        "pc2": rng.standard_normal((B, N, 3), np.float32),
        "feat1": rng.standard_normal((B, N, D), np.float32),
        "feat2": rng.standard_normal((B, N, D), np.float32),
        "pos_w": (rng.standard_normal((D, 3)) * 0.1).astype(np.float32),
        "pos_b": (rng.standard_normal((D,)) * 0.1).astype(np.float32),
        "mlp_w0": (rng.standard_normal((D, D)) * 0.1).astype(np.float32),
        "mlp_b0": (rng.standard_normal((D,)) * 0.1).astype(np.float32),
        "mlp_w1": (rng.standard_normal((D, D)) * 0.1).astype(np.float32),
        "mlp_b1": (rng.standard_normal((D,)) * 0.1).astype(np.float32),
        "t1_w": (rng.standard_normal((128, D)) * 0.1).astype(np.float32),
        "t1_b": (rng.standard_normal((128,)) * 0.1).astype(np.float32),
        "t2_w": (rng.standard_normal((128, D)) * 0.1).astype(np.float32),
        "t2_b": (rng.standard_normal((128,)) * 0.1).astype(np.float32),
    }
    f1, f2 = kernel(**ins)
    print("f1", f1.shape, "f2", f2.shape)


# revision 15
# speedup vs baseline: 2.6143x; 2.6143x over previous
"""Trainium2 Bass kernel for nn_CrossLayerLight (cross-cloud KNN message passing).

Sharding: 8 cores = 2 directions x 2 batches x 2 query-halves.
Each core: 4096 queries vs 8192 candidates.

Round-3 design: Morton-sorted queries and candidates; candidates in 64
Morton-contiguous groups of 128.  Per 128-query tile the device selects the
top-16 groups by a coarse bound (-(d(q,ctr_g)-r_g)^2 maxed over the tile's
queries, computed once for all tiles in phase C0), gathers those groups'
score-table rows (2048 candidates), and runs exact top-16 on the 2048-wide
window only.  On this dataset each tile's true top-16 neighbors span <= 8
groups, so 16 give 2x margin (validated offline).

Per-core device pipeline:
  A) v-table build: v[j] = feat2[j] + xyz2[j] @ pos_w^T -> bf16 hi/lo pair
     rows [8192, 128] in DRAM for gathering
  B) u-table build: u[q] = feat1[q] - xyz1[q] @ pos_w^T + pos_b
  C0) coarse group stage: d2(q, ctr_g) via 32-row bf16 matmul, sqrt (ACT),
      lb = -relu(d-r)^2, priority = partition-all-reduce max (GPSIMD),
      per-tile top-16 groups + ids (DVE max8/max_index)
  C) per 128-query tile:
     - gather the 16 selected groups' score-table rows (256B each) ->
       [128p, 2048] bf16 rhs; scores via 30-row bf16 matmul (PE, ~1e-6)
     - exact top-16 of the 2048 window: ACT copies PSUM->SBUF, then DVE
       max8/max_index/match_replace/max8/max_index (window positions)
     - window pos -> global candidate id: id = g16[w>>7]*128 + (w&127) with
       the 16-entry group lookup done by a one-hot multiply-reduce
     - idx transpose+replicate via PE transpose -> dma_gather of v-pairs
     - z0/L0 relu pairs fused via output-stacked weights; L1 -> max-pool over
       k from PSUM (DVE); final 64->128 linear with fused bias (PE), DMA out.
"""

import sys
import numpy as np
import ml_dtypes

sys.path.insert(0, "/opt/trn_rl_repo")

import concourse.bacc as bacc  # noqa: E402
import concourse.mybir as mybir  # noqa: E402
import concourse.bass_isa as bass_isa  # noqa: E402
from concourse.bass_utils import run_bass_kernel_spmd  # noqa: E402
from concourse.tile import TileContext  # noqa: E402

BF16 = ml_dtypes.bfloat16
F32 = mybir.dt.float32
F32R = mybir.dt.float32r
BF = mybir.dt.bfloat16
U32 = mybir.dt.uint32
I16 = mybir.dt.int16

NQ_TOT = 4096   # queries per core
NCAND = 8192    # candidates per core
D = 64          # feature dim
KNN = 16
NROW = 30       # score matmul contraction rows
NGRP = 64       # candidate groups (128 each)
GSEL = 16       # groups selected per query tile -> window of 2048
WSEL = GSEL * 128
LEAKY = 0.1
MBITS = 10

_CACHE = {}


def build_nc(nq_tot=NQ_TOT, ncand=NCAND, body_repeats=1,
             no_scog=False, no_vg=False):
    nc = bacc.Bacc()
    ntiles = nq_tot // 128

    # ---- external inputs ----
    sc_lhsT = nc.dram_tensor("sc_lhsT", [NROW, nq_tot], BF, kind="ExternalInput")
    scotab = nc.dram_tensor("scotab", [ncand, 128], BF, kind="ExternalInput")
    cg_lhsT = nc.dram_tensor("cg_lhsT", [34, nq_tot], BF, kind="ExternalInput")
    cg_rhs = nc.dram_tensor("cg_rhs", [34, NGRP], BF, kind="ExternalInput")
    rg_in = nc.dram_tensor("rg_in", [128, NGRP], F32, kind="ExternalInput")
    vb_lhsT = nc.dram_tensor("vb_lhsT", [67, ncand], F32, kind="ExternalInput")
    vb_rhs = nc.dram_tensor("vb_rhs", [67, D], F32, kind="ExternalInput")
    ub_lhsT = nc.dram_tensor("ub_lhsT", [68, D], F32, kind="ExternalInput")
    ub_rhs = nc.dram_tensor("ub_rhs", [68, nq_tot], F32, kind="ExternalInput")
    w0T = nc.dram_tensor("w0T", [2 * D, 2 * D], F32R, kind="ExternalInput")
    w1T = nc.dram_tensor("w1T", [2 * D, D], F32R, kind="ExternalInput")
    bb0 = nc.dram_tensor("bb0", [2 * D, 1], F32, kind="ExternalInput")
    b1c = nc.dram_tensor("b1c", [D, 1], F32, kind="ExternalInput")
    b1n = nc.dram_tensor("b1n", [D, 1], F32, kind="ExternalInput")
    t_rhs = nc.dram_tensor("t_rhs", [2 * D, 128], F32R, kind="ExternalInput")
    tb_row = nc.dram_tensor("tb_row", [1, 128], F32, kind="ExternalInput")
    ii128 = nc.dram_tensor("ii128", [2 * D, 2 * D], BF, kind="ExternalInput")
    id128u = nc.dram_tensor("id128u", [128, 128], F32, kind="ExternalInput")
    gpat_in = nc.dram_tensor("gpat_in", [128, 128], F32, kind="ExternalInput")
    iota_in = nc.dram_tensor("iota_in", [128, GSEL], F32, kind="ExternalInput")
    shc_in = nc.dram_tensor("shc_in", [128, 4], U32, kind="ExternalInput")

    out = nc.dram_tensor("out", [nq_tot, 128], F32, kind="ExternalOutput")

    with TileContext(nc) as tc:
        with (
            tc.tile_pool(name="const", bufs=1) as cst,
            tc.tile_pool(name="dram", bufs=1, space="DRAM") as dram,
        ):
            vpair = dram.tile([ncand, 128], BF)

            # persistent SBUF tiles
            sc_l = cst.tile([NROW, nq_tot], BF)
            cgl = cst.tile([34, nq_tot], BF)
            cgr = cst.tile([34, NGRP], BF)
            rgs = cst.tile([128, NGRP], F32)
            uhl = cst.tile([128, nq_tot], BF)      # rows 0:64 u_hi, 64:128 u_lo
            w0s = cst.tile([2 * D, 2 * D], F32R)
            w1s = cst.tile([2 * D, D], F32R)
            bb0s = cst.tile([2 * D, 1], F32)
            b1s = cst.tile([D, 1], F32)
            b1ns = cst.tile([D, 1], F32)
            trs = cst.tile([2 * D, 128], F32R)
            tbs = cst.tile([1, 128], F32)
            ones1 = cst.tile([1, 128], F32)
            iis = cst.tile([2 * D, 2 * D], BF)
            idu = cst.tile([128, 128], F32)
            gpatc = cst.tile([128, 128], F32)
            iotac = cst.tile([128, GSEL], F32)
            shc = cst.tile([128, 4], U32)          # cols: [7, 127, 0, 0]
            g16all = cst.tile([128, ntiles * GSEL], F32)
            for dst, src in [(sc_l, sc_lhsT), (cgl, cg_lhsT), (cgr, cg_rhs),
                             (rgs, rg_in), (w0s, w0T), (w1s, w1T), (bb0s, bb0),
                             (b1s, b1c), (b1ns, b1n), (trs, t_rhs), (tbs, tb_row),
                             (iis, ii128), (idu, id128u), (gpatc, gpat_in),
                             (iotac, iota_in), (shc, shc_in)]:
                nc.sync.dma_start(out=dst[:], in_=src[:])
            nc.vector.memset(ones1[:], 1.0)

            for _body_rep in range(body_repeats):
                # ---- phase A: v table ----
                with (
                    tc.tile_pool(name="phA", bufs=2) as pha,
                    tc.tile_pool(name="phA_ps", bufs=2, space="PSUM") as phaps,
                ):
                    vbw = pha.tile([67, D], F32, tag="vbw")
                    nc.sync.dma_start(out=vbw[:], in_=vb_rhs[:])
                    njt = ncand // 128
                    grp = 8  # j-tiles per psum fill
                    for g in range(njt // grp):
                        pv = phaps.tile([128, grp * D], F32, tag="pv")
                        for s in range(grp):
                            jt = g * grp + s
                            vbl = pha.tile([67, 128], F32, tag="vbl")
                            nc.sync.dma_start(out=vbl[:], in_=vb_lhsT[:, jt * 128:(jt + 1) * 128])
                            nc.tensor.matmul(pv[:, s * D:(s + 1) * D], vbl[:], vbw[:],
                                             start=True, stop=True)
                        vhi = pha.tile([128, grp * D], BF, tag="vhi")
                        vlo = pha.tile([128, grp * D], BF, tag="vlo")
                        nc.scalar.activation(vhi[:], pv[:], mybir.ActivationFunctionType.Copy)
                        nc.vector.tensor_sub(vlo[:], pv[:], vhi[:])
                        dst = vpair[g * grp * 128:(g + 1) * grp * 128, :]
                        dst_hi = dst[:, 0:D].rearrange("(s p) f -> p s f", p=128)
                        dst_lo = dst[:, D:128].rearrange("(s p) f -> p s f", p=128)
                        nc.sync.dma_start(out=dst_hi, in_=vhi[:].rearrange("p (s f) -> p s f", f=D))
                        nc.sync.dma_start(out=dst_lo, in_=vlo[:].rearrange("p (s f) -> p s f", f=D))

                # ---- phase B: u table ----
                with (
                    tc.tile_pool(name="phB", bufs=2) as phb,
                    tc.tile_pool(name="phB_ps", bufs=2, space="PSUM") as phbps,
                ):
                    ubw = phb.tile([68, D], F32, tag="ubw")
                    nc.sync.dma_start(out=ubw[:], in_=ub_lhsT[:])
                    uchunk = min(2048, nq_tot)
                    for h in range(nq_tot // uchunk):
                        ur = phb.tile([68, uchunk], F32, tag="ur")
                        nc.sync.dma_start(out=ur[:], in_=ub_rhs[:, h * uchunk:(h + 1) * uchunk])
                        pu = phbps.tile([D, uchunk], F32, tag="pu")
                        for j in range(uchunk // 512 or 1):
                            w = min(512, uchunk)
                            nc.tensor.matmul(pu[:, j * w:(j + 1) * w], ubw[:],
                                             ur[:, j * w:(j + 1) * w], start=True, stop=True)
                        nc.scalar.activation(uhl[0:D, h * uchunk:(h + 1) * uchunk], pu[:],
                                             mybir.ActivationFunctionType.Copy)
                        nc.vector.tensor_sub(uhl[D:128, h * uchunk:(h + 1) * uchunk], pu[:],
                                             uhl[0:D, h * uchunk:(h + 1) * uchunk])

                # ---- phase C ----
                with (
                    tc.tile_pool(name="wk", bufs=2) as wk,
                    tc.tile_pool(name="cw", bufs=1) as cw,
                    tc.tile_pool(name="ps_sc", bufs=2, space="PSUM") as pssc,
                    tc.tile_pool(name="ps_z", bufs=2, space="PSUM") as psz,
                    tc.tile_pool(name="ps_tr", bufs=1, space="PSUM") as pstr,
                    tc.tile_pool(name="ps_t1", bufs=1, space="PSUM") as pst1,
                ):
                    # C0: coarse group stage for all tiles
                    dall = cw.tile([128, ntiles * NGRP], F32)   # d(q, ctr_g)
                    nlb = cw.tile([128, ntiles * NGRP], F32)
                    for half in range(2):
                        pst = pssc.tile([128, 1024], F32, tag="psc")
                        for tt in range(16):
                            t = half * 16 + tt
                            nc.tensor.matmul(pst[:, tt * NGRP:(tt + 1) * NGRP],
                                             cgl[:, t * 128:(t + 1) * 128], cgr[:],
                                             start=True, stop=True)
                        nc.scalar.activation(dall[:, half * 1024:(half + 1) * 1024],
                                             pst[:], mybir.ActivationFunctionType.Sqrt)
                    rgb = rgs[:].unsqueeze(1).to_broadcast([128, ntiles, NGRP])
                    d3 = dall[:].rearrange("p (t g) -> p t g", g=NGRP)
                    nlb3 = nlb[:].rearrange("p (t g) -> p t g", g=NGRP)
                    nc.vector.tensor_tensor(out=nlb3, in0=d3, in1=rgb,
                                            op=mybir.AluOpType.subtract)
                    nc.vector.tensor_scalar(nlb[:], nlb[:], 0.0, scalar2=None,
                                            op0=mybir.AluOpType.max)
                    nc.vector.scalar_tensor_tensor(
                        out=nlb[:], in0=nlb[:], scalar=-1.0, in1=nlb[:],
                        op0=mybir.AluOpType.mult, op1=mybir.AluOpType.mult)
                    nc.gpsimd.partition_all_reduce(out_ap=nlb[:], in_ap=nlb[:],
                                                   channels=128,
                                                   reduce_op=bass_isa.ReduceOp.max)
                    g8t = cw.tile([128, 16], F32)
                    gpos = cw.tile([128, GSEL], U32)
                    for t in range(ntiles):
                        sl_n = nlb[:, t * NGRP:(t + 1) * NGRP]
                        nc.vector.max(out=g8t[:, 0:8], in_=sl_n)
                        nc.vector.max_index(out=gpos[:, 0:8], in_max=g8t[:, 0:8],
                                            in_values=sl_n)
                        nc.vector.match_replace(out=sl_n, in_to_replace=g8t[:, 0:8],
                                                in_values=sl_n, imm_value=-3.0e38)
                        nc.vector.max(out=g8t[:, 8:16], in_=sl_n)
                        nc.vector.max_index(out=gpos[:, 8:16], in_max=g8t[:, 8:16],
                                            in_values=sl_n)
                        nc.vector.tensor_copy(g16all[:, t * GSEL:(t + 1) * GSEL],
                                              gpos[:])

                    # C per tile
                    for qt in range(ntiles):
                        q0 = qt * 128
                        g16f = g16all[:, qt * GSEL:(qt + 1) * GSEL]
                        # window gather: idx[p,c] = g16[c>>3]*128 + 16*(c&7) + p%16
                        gA = wk.tile([128, 128], F32, tag="gA")
                        nc.vector.tensor_copy(
                            gA[:].rearrange("p (a b) -> p a b", b=8),
                            g16f.unsqueeze(2).to_broadcast([128, GSEL, 8]))
                        idst = wk.tile([128, 128], F32, tag="idst")
                        nc.vector.scalar_tensor_tensor(
                            out=idst[:], in0=gA[:], scalar=128.0, in1=gpatc[:],
                            op0=mybir.AluOpType.mult, op1=mybir.AluOpType.add)
                        idxs2 = wk.tile([128, 128], I16, tag="idxs2")
                        nc.vector.tensor_copy(idxs2[:], idst[:])
                        srhs = wk.tile([128, 1, WSEL], BF, tag="srhs")
                        if no_scog:
                            nc.vector.memset(srhs[:], 0.0)
                        else:
                            nc.gpsimd.dma_gather(out_ap=srhs[:], in_ap=scotab[:],
                                                 idxs_ap=idxs2[:], num_idxs=WSEL,
                                                 num_idxs_reg=WSEL, elem_size=128,
                                                 transpose=True, single_packet=False)
                        # scores on the window
                        wsb = wk.tile([128, WSEL], F32, tag="wsb")
                        for b2 in range(WSEL // 1024):
                            pst = pssc.tile([128, 1024], F32, tag="psc")
                            for j in range(2):
                                c0 = b2 * 1024 + j * 512
                                nc.tensor.matmul(pst[:, j * 512:(j + 1) * 512],
                                                 sc_l[:, q0:q0 + 128],
                                                 srhs[0:NROW, 0, c0:c0 + 512],
                                                 start=True, stop=True)
                            nc.scalar.activation(wsb[:, b2 * 1024:(b2 + 1) * 1024],
                                                 pst[:], mybir.ActivationFunctionType.Copy)
                        # exact top-16 of the window (positions = window idx)
                        r16 = wk.tile([128, 16], F32, tag="r16")
                        pos = wk.tile([128, 16], U32, tag="pos")
                        nc.vector.max(out=r16[:, 0:8], in_=wsb[:])
                        nc.vector.max_index(out=pos[:, 0:8], in_max=r16[:, 0:8],
                                            in_values=wsb[:])
                        nc.vector.match_replace(out=wsb[:], in_to_replace=r16[:, 0:8],
                                                in_values=wsb[:], imm_value=-3.0e38)
                        nc.vector.max(out=r16[:, 8:16], in_=wsb[:])
                        nc.vector.max_index(out=pos[:, 8:16], in_max=r16[:, 8:16],
                                            in_values=wsb[:])
                        # window pos -> global candidate id
                        gidx = wk.tile([128, 16], U32, tag="gidx")
                        rem = wk.tile([128, 16], U32, tag="rem")
                        nc.vector.tensor_scalar(gidx[:], pos[:], shc[:, 0:1],
                                                scalar2=None,
                                                op0=mybir.AluOpType.logical_shift_right)
                        nc.vector.tensor_scalar(rem[:], pos[:], shc[:, 1:2],
                                                scalar2=None,
                                                op0=mybir.AluOpType.bitwise_and)
                        gf = wk.tile([128, 16], F32, tag="gf")
                        remf = wk.tile([128, 16], F32, tag="remf")
                        nc.vector.tensor_copy(gf[:], gidx[:])
                        nc.vector.tensor_copy(remf[:], rem[:])
                        eqm = wk.tile([128, 16, GSEL], F32, tag="eqm")
                        nc.vector.tensor_tensor(
                            out=eqm[:],
                            in0=gf[:].unsqueeze(2).to_broadcast([128, 16, GSEL]),
                            in1=iotac[:].unsqueeze(1).to_broadcast([128, 16, GSEL]),
                            op=mybir.AluOpType.is_equal)
                        nc.vector.tensor_tensor(
                            out=eqm[:], in0=eqm[:],
                            in1=g16f.unsqueeze(1).to_broadcast([128, 16, GSEL]),
                            op=mybir.AluOpType.mult)
                        gbase = wk.tile([128, 16], F32, tag="gbase")
                        nc.vector.tensor_reduce(out=gbase[:], in_=eqm[:],
                                                axis=mybir.AxisListType.X,
                                                op=mybir.AluOpType.add)
                        idxf = wk.tile([128, 16], F32, tag="idxf")
                        nc.vector.scalar_tensor_tensor(
                            out=idxf[:], in0=gbase[:], scalar=128.0, in1=remf[:],
                            op0=mybir.AluOpType.mult, op1=mybir.AluOpType.add)
                        # C3: idx -> replicate x8 in free dim -> PE transpose
                        i16r = wk.tile([128, 128], F32, tag="i16r")
                        rep = idxf[:].unsqueeze(1).to_broadcast([128, 8, 16])
                        nc.vector.tensor_copy(i16r[:].rearrange("p (r k) -> p r k", k=16), rep)
                        ptr = pstr.tile([128, 128], F32, tag="tr")
                        nc.tensor.transpose(ptr[:], i16r[:], idu[:])
                        idxs = wk.tile([128, 128], I16, tag="idxs")
                        nc.vector.tensor_copy(idxs[:], ptr[:])
                        # C4: gather v pairs -> [128, 2048] bf16 (cols q*16+k)
                        gt = wk.tile([128, 1, 2048], BF, tag="gt")
                        if no_vg:
                            nc.vector.memset(gt[:], 0.0)
                        else:
                            nc.gpsimd.dma_gather(out_ap=gt[:], in_ap=vpair[:], idxs_ap=idxs[:],
                                                 num_idxs=2048, num_idxs_reg=2048,
                                                 elem_size=128, transpose=True,
                                                 single_packet=False)
                        gtf = gt[:].rearrange("p a n -> p (a n)")
                        # C5-C8 per 512-col block (32 queries): z0/L0 emit [y; -y]
                        pooled = wk.tile([D, 128], F32, tag="pooled")
                        for cb in range(4):
                            sl = slice(cb * 512, (cb + 1) * 512)
                            pz0 = psz.tile([128, 512], F32, tag="pz")
                            nc.tensor.matmul(pz0[:], iis[:], gtf[:, sl],
                                             start=True, stop=False)
                            urhs = uhl[:, q0 + cb * 32:q0 + (cb + 1) * 32] \
                                .unsqueeze(2).to_broadcast([128, 32, KNN])
                            nc.tensor.matmul(pz0[:].rearrange("p (q k) -> p q k", k=KNN),
                                             iis[:], urhs, start=False, stop=True)
                            rp0 = wk.tile([2 * D, 512], F32R, tag="rp")
                            nc.scalar.activation(rp0[:], pz0[:],
                                                 mybir.ActivationFunctionType.Relu)
                            pz1 = psz.tile([128, 512], F32, tag="pz")
                            nc.tensor.matmul(pz1[:], w0s[:], rp0[:], start=True, stop=True)
                            rp1 = wk.tile([2 * D, 512], F32R, tag="rp")
                            nc.scalar.activation(rp1[:], pz1[:],
                                                 mybir.ActivationFunctionType.Relu,
                                                 bias=bb0s[:])
                            pz2 = psz.tile([D, 512], F32, tag="pz")
                            nc.tensor.matmul(pz2[:], w1s[:], rp1[:], start=True, stop=True)
                            nc.vector.tensor_reduce(
                                out=pooled[:, cb * 32:(cb + 1) * 32],
                                in_=pz2[:].rearrange("p (q k) -> p q k", k=KNN),
                                axis=mybir.AxisListType.X, op=mybir.AluOpType.max)
                        # t-linear
                        tl = wk.tile([2 * D, 128], F32R, tag="tl")
                        nc.scalar.activation(tl[0:D, :], pooled[:],
                                             mybir.ActivationFunctionType.Relu, bias=b1s[:])
                        nc.scalar.activation(tl[D:2 * D, :], pooled[:],
                                             mybir.ActivationFunctionType.Relu,
                                             bias=b1ns[:], scale=-1.0)
                        pt1 = pst1.tile([128, 128], F32, tag="t1")
                        nc.tensor.matmul(pt1[:], tl[:], trs[:], start=True, stop=False)
                        nc.tensor.matmul(pt1[:], ones1[:], tbs[:], start=False, stop=True)
                        outt = wk.tile([128, 128], F32, tag="outt")
                        nc.scalar.activation(outt[:], pt1[:], mybir.ActivationFunctionType.Copy)
                        nc.sync.dma_start(out=out[q0:q0 + 128, :], in_=outt[:])

    nc.compile()
    return nc


def _split_bf16(x, n):
    parts = []
    rem = np.asarray(x, np.float64)
    for _ in range(n):
        p = rem.astype(BF16)
        parts.append(p)
        rem = rem - p.astype(np.float64)
    return parts


def _morton_key(cand, pts):
    n = len(cand)
    q = np.zeros((len(pts), 3), np.uint64)
    for c in range(3):
        sc = np.sort(cand[:, c])
        q[:, c] = np.clip(np.searchsorted(sc, pts[:, c]) * (1 << MBITS) // (n + 1),
                          0, (1 << MBITS) - 1)
    k = np.zeros(len(pts), np.uint64)
    for b in range(MBITS):
        for c in range(3):
            k |= ((q[:, c] >> np.uint64(b)) & np.uint64(1)) << np.uint64(3 * b + c)
    return k


def _score_rows(qxyz, cxyz, negate_c=False, norm_sign=-1.0, norm_of="c"):
    """30-row bf16 3-term-split factorization of 2 q.p + norm_sign*|p|^2."""
    nq = qxyz.shape[0]
    A = _split_bf16(2.0 * qxyz, 3)
    P = _split_bf16(cxyz, 3)
    nrm = np.sum(np.asarray(cxyz, np.float64) ** 2, -1)
    m = _split_bf16(norm_sign * nrm, 3)
    rows_q, rows_c = [], []
    prods = sorted(((i, j) for i in range(3) for j in range(3)),
                   key=lambda t: -(t[0] + t[1]))
    sgn = -1.0 if negate_c else 1.0
    for (i, j) in prods:
        for c in range(3):
            rows_q.append(A[i][:, c])
            rows_c.append(sgn * P[j][:, c])
    ones = np.ones(nq, BF16)
    for t in (m[2], m[1], m[0]):
        rows_q.append(ones)
        rows_c.append(t)
    return np.stack(rows_q).astype(BF16), np.stack(rows_c).astype(BF16)


def prep_core_inputs(qxyz, qfeat, cxyz, cfeat, pos_w, pos_b, tw, tb):
    """Per-core input map. Host work is O(N log N) sort + O(N*small) layout."""
    # Morton sort both clouds along the candidate-cloud quantile curve
    cperm = np.argsort(_morton_key(cxyz, cxyz), kind="stable")
    qperm = np.argsort(_morton_key(cxyz, qxyz), kind="stable")
    qxyz, qfeat = qxyz[qperm], qfeat[qperm]
    cxyz, cfeat = cxyz[cperm], cfeat[cperm]
    nq = qxyz.shape[0]
    ncand = cxyz.shape[0]

    sc_lhsT, sc_rhs = _score_rows(qxyz, cxyz)    # [30, nq], [30, ncand]
    scotab = np.zeros((ncand, 128), BF16)
    scotab[:, 0:NROW] = sc_rhs.T

    # groups: bbox centers + radii
    cg = cxyz.reshape(NGRP, 128, 3).astype(np.float64)
    ctr = (cg.min(1) + cg.max(1)) / 2                       # [G,3]
    rad = np.sqrt(((cg - ctr[:, None, :]) ** 2).sum(-1)).max(1)  # [G]

    # coarse d2 matmul: d2 = |q|^2 - 2 q.c + |c|^2
    cq_rows, cc_rows = _score_rows(qxyz, ctr, negate_c=True, norm_sign=1.0)
    q2 = _split_bf16(np.sum(qxyz.astype(np.float64) ** 2, -1), 3)
    onesq = np.ones(nq, BF16)
    onesg = np.ones(NGRP, BF16)
    eps_row = np.full(NGRP, 1e-5, BF16)
    cg_lhsT = np.concatenate([cq_rows, np.stack([q2[0], q2[1], q2[2], onesq])]).astype(BF16)
    cg_rhs = np.concatenate([cc_rows, np.stack([onesg, onesg, onesg, eps_row])]).astype(BF16)
    rg_in = np.broadcast_to(rad.astype(np.float32), (128, NGRP)).copy()

    vb_lhsT = np.concatenate([cxyz.T, cfeat.T]).astype(np.float32)       # [67, ncand]
    vb_rhs = np.concatenate([pos_w.T, np.eye(D)]).astype(np.float32)     # [67, 64]
    ub_lhsT = np.concatenate([-pos_w.T, np.eye(D), pos_b[None, :]]).astype(np.float32)
    ub_rhs = np.concatenate([qxyz.T, qfeat.T, np.ones((1, nq))]).astype(np.float32)

    t_rhs = np.concatenate([tw.T, -LEAKY * tw.T]).astype(np.float32)     # [128, 128]
    tb_row = tb[None, :].astype(np.float32)
    ii1 = np.concatenate([np.eye(D), np.eye(D)])                         # [128, 64]
    ii = np.concatenate([ii1, -ii1], axis=1).astype(BF16)                # [128, 128]
    idu = np.eye(128).astype(np.float32)

    # gather idx pattern: gpat[p, c] = 16*(c&7) + p%16
    pp = (np.arange(128) % 16)[:, None]
    cc = (np.arange(128) & 7)[None, :] * 16
    gpat = (pp + cc).astype(np.float32)
    iota = np.broadcast_to(np.arange(GSEL, dtype=np.float32), (128, GSEL)).copy()
    shc = np.broadcast_to(np.array([7, 127, 0, 0], np.uint32), (128, 4)).copy()

    return {
        "sc_lhsT": sc_lhsT, "scotab": scotab,
        "cg_lhsT": cg_lhsT, "cg_rhs": cg_rhs, "rg_in": rg_in,
        "vb_lhsT": vb_lhsT, "vb_rhs": vb_rhs,
        "ub_lhsT": ub_lhsT, "ub_rhs": ub_rhs,
        "w0T": None, "w1T": None, "bb0": None,  # filled by caller (shared)
        "b1c": None, "b1n": None,
        "t_rhs": t_rhs, "tb_row": tb_row, "ii128": ii, "id128u": idu,
        "gpat_in": gpat, "iota_in": iota, "shc_in": shc,
        "_qperm": qperm,
    }


def build_in_maps(inputs):
    pc1 = np.asarray(inputs["pc1"]); pc2 = np.asarray(inputs["pc2"])
    feat1 = np.asarray(inputs["feat1"]); feat2 = np.asarray(inputs["feat2"])
    pos_w = np.asarray(inputs["pos_w"]); pos_b = np.asarray(inputs["pos_b"])
    w0 = np.asarray(inputs["mlp_w0"]); b0 = np.asarray(inputs["mlp_b0"])
    w1 = np.asarray(inputs["mlp_w1"]); b1 = np.asarray(inputs["mlp_b1"])
    t1w = np.asarray(inputs["t1_w"]); t1b = np.asarray(inputs["t1_b"])
    t2w = np.asarray(inputs["t2_w"]); t2b = np.asarray(inputs["t2_b"])

    w0a = np.concatenate([w0.T, -LEAKY * w0.T])                  # [128, 64]
    w0T = np.concatenate([w0a, -w0a], axis=1).astype(np.float32)  # [128, 128]
    w1T = np.concatenate([w1.T, -LEAKY * w1.T]).astype(np.float32)
    bb0 = np.concatenate([b0, -b0]).astype(np.float32)[:, None].copy()
    b1c = b1.astype(np.float32)[:, None].copy()

    half = NQ_TOT
    in_maps = []
    core_meta = []
    for d in range(2):
        for b in range(2):
            for h in range(2):
                if d == 0:
                    q, p, fq, fp, tw, tb = pc1[b], pc2[b], feat1[b], feat2[b], t1w, t1b
                else:
                    q, p, fq, fp, tw, tb = pc2[b], pc1[b], feat2[b], feat1[b], t2w, t2b
                sl = slice(h * half, (h + 1) * half)
                m = prep_core_inputs(q[sl], fq[sl], p, fp, pos_w, pos_b, tw, tb)
                m["w0T"] = w0T; m["w1T"] = w1T; m["bb0"] = bb0
                m["b1c"] = b1c; m["b1n"] = -b1c
                qperm = m.pop("_qperm")
                in_maps.append(m)
                core_meta.append((d, b, h, qperm))
    return in_maps, core_meta


def kernel(pc1, pc2, feat1, feat2, pos_w, pos_b, mlp_w0, mlp_b0,
           mlp_w1, mlp_b1, t1_w, t1_b, t2_w, t2_b, _trace=False):
    pc1 = np.asarray(pc1)

    if "nc" not in _CACHE:
        _CACHE["nc"] = build_nc()
    nc = _CACHE["nc"]

    inputs = dict(pc1=pc1, pc2=pc2, feat1=feat1, feat2=feat2, pos_w=pos_w,
                  pos_b=pos_b, mlp_w0=mlp_w0, mlp_b0=mlp_b0, mlp_w1=mlp_w1,
                  mlp_b1=mlp_b1, t1_w=t1_w, t1_b=t1_b, t2_w=t2_w, t2_b=t2_b)
    in_maps, core_meta = build_in_maps(inputs)
    _CACHE["last_in_maps"] = in_maps

    res = run_bass_kernel_spmd(nc, in_maps, core_ids=list(range(8)), trace=_trace)
    _CACHE["last_res"] = res
    half = NQ_TOT

    B, N = pc1.shape[0], pc1.shape[1]
    f1 = np.zeros((B, N, 128), np.float32)
    f2 = np.zeros((B, N, 128), np.float32)
    for (dd, b, h, qperm), r in zip(core_meta, res.results):
        o = r["out"]
        tgt = f1 if dd == 0 else f2
        tgt[b, h * half + qperm, :] = o
    return f1, f2


if __name__ == "__main__":
    rng = np.random.default_rng(0)
    B, N = 2, 8192
    ins = {
        "pc1": rng.standard_normal((B, N, 3), np.float32),
        "pc2": rng.standard_normal((B, N, 3), np.float32),
        "feat1": rng.standard_normal((B, N, D), np.float32),
        "feat2": rng.standard_normal((B, N, D), np.float32),
        "pos_w": (rng.standard_normal((D, 3)) * 0.1).astype(np.float32),
        "pos_b": (rng.standard_normal((D,)) * 0.1).astype(np.float32),
        "mlp_w0": (rng.standard_normal((D, D)) * 0.1).astype(np.float32),
        "mlp_b0": (rng.standard_normal((D,)) * 0.1).astype(np.float32),
        "mlp_w1": (rng.standard_normal((D, D)) * 0.1).astype(np.float32),
        "mlp_b1": (rng.standard_normal((D,)) * 0.1).astype(np.float32),
        "t1_w": (rng.standard_normal((128, D)) * 0.1).astype(np.float32),
        "t1_b": (rng.standard_normal((128,)) * 0.1).astype(np.float32),
        "t2_w": (rng.standard_normal((128, D)) * 0.1).astype(np.float32),
        "t2_b": (rng.standard_normal((128,)) * 0.1).astype(np.float32),
    }
    f1, f2 = kernel(**ins)
    print("f1", f1.shape, "f2", f2.shape)
